# revision 2
# baseline (speedup 1.0000x reference)
"""ConsciousMoE kernel for 8 Trainium2 NeuronCores.

Reference computation (all fp32):
    c       = mean(states, axis=0)                     # [H=2048]
    w       = softmax(c @ Wr + br)                     # [E=16]
    vals,i  = top_k(w, 2); vals /= vals.sum()
    h       = gelu(c @ Wup[i] + bup[i])                # [2, EH=4096]
    eo      = h @ Wdown[i] + bdown[i]                  # [2, V=4096]
    out     = vals @ eo                                # [V=4096]

Sharding: every core computes the (tiny) routing redundantly; Wup is sharded
along EXP_HID (each core owns a 512-wide slice of every expert's Wup columns
and the matching 512 rows of Wdown).  Each core computes its partial
vals-weighted expert output over the full vocab and an 8-way AllReduce sums
the partials.  Per-core HBM traffic for the two selected experts:
2*(2048*512 + 512*4096)*4B = 24 MB (1/8 of the 192 MB the reference touches).

Expert selection is data-dependent: the kernel computes top-2 on device
(vector.max / max_index), converts the indices to engine registers
(values_load) and issues dynamic-offset DMAs (bass.ds / bass.ts) so only the
two selected experts' weight shards are ever read from HBM.
"""

import numpy as np

N_EXPERTS = 16
TOP_K = 2
HIDDEN = 2048
EXP_HID = 4096
VOCAB = 4096
CELLS = 64
N_CORES = 8
SHARD = EXP_HID // N_CORES          # 512 columns of Wup / rows of Wdown per core
P = 128                             # SBUF partitions
HC = HIDDEN // P                    # 16 hidden chunks of 128
SJ = SHARD // P                     # 4 shard sub-chunks of 128
NB = VOCAB // 512                   # 8 PSUM bank regions of 512

CHUNK = 2048                        # 1 MB weight chunks: [128, 2048] fp32
WUP_CHUNKS = (HC * SHARD) // CHUNK  # 4 per expert
WDN_CHUNKS = (SJ * VOCAB) // CHUNK  # 8 per expert

_CACHE = {}


def _emit_body(nc, tc, tensors, rep):
    """One full MoE forward; `rep` only uniquifies PSUM pool names."""
    import concourse.bass as bass
    import concourse.mybir as mybir

    f32 = mybir.dt.float32
    ET = mybir.EngineType
    AX = mybir.AxisListType
    AF = mybir.ActivationFunctionType

    (statesT_d, wr_d, br_d, wup_d, wdown_d, bup_d, bdown_d, out_d) = tensors["dram"]
    dbg_d = tensors.get("dbg")
    cpool, rpool, wpool, hpool, dpool = tensors["pools"]

    psr_ctx = tc.tile_pool(name=f"psr{rep}", bufs=2, space="PSUM")
    psr = psr_ctx.__enter__()

    # ---------- phase 0: c = mean(states) ----------
    statesT = cpool.tile([P, HC * CELLS], f32, name=f"statesT{rep}")
    nc.sync.dma_start(statesT[:, :], statesT_d[:, :])
    wr_sb = cpool.tile([P, HC * N_EXPERTS], f32, name=f"wr_sb{rep}")
    nc.sync.dma_start(wr_sb[:, :], wr_d[:, :])
    br_sb = rpool.tile([1, N_EXPERTS], f32, name=f"br_sb{rep}")
    nc.sync.dma_start(br_sb[:, :], br_d[:, :])

    c_sb = cpool.tile([P, HC], f32, name=f"c_sb{rep}")
    for c in range(HC):
        nc.vector.reduce_sum(
            c_sb[:, c : c + 1], statesT[:, c * CELLS : (c + 1) * CELLS], AX.X
        )
    nc.scalar.mul(c_sb[:, :], c_sb[:, :], 1.0 / CELLS)

    # ---------- phase 1: router logits ----------
    plog = psr.tile([1, N_EXPERTS], f32, name=f"plog{rep}", tag="ps_small")
    for c in range(HC):
        nc.tensor.matmul(
            plog[:, :],
            c_sb[:, c : c + 1],
            wr_sb[:, c * N_EXPERTS : (c + 1) * N_EXPERTS],
            start=(c == 0),
            stop=(c == HC - 1),
        )
    logits = rpool.tile([1, N_EXPERTS], f32, name=f"logits{rep}")
    nc.vector.tensor_add(logits[:, :], plog[:, :], br_sb[:, :])

    # ---------- phase 2: top-2 ----------
    max8 = rpool.tile([1, 8], f32, name=f"max8{rep}")
    idx8 = rpool.tile([1, 8], mybir.dt.uint32, name=f"idx8{rep}")
    nc.vector.max(max8[:, :], logits[:, :])
    nc.vector.max_index(idx8[:, :], max8[:, :], logits[:, :])

    # vals = softmax over the two top logits (full-softmax denom cancels)
    dlt = rpool.tile([1, 1], f32, name=f"dlt{rep}")
    nc.vector.tensor_sub(dlt[:, :], max8[:, 1:2], max8[:, 0:1])
    ex = rpool.tile([1, 1], f32, name=f"ex{rep}")
    nc.scalar.activation(ex[:, :], dlt[:, :], AF.Exp)
    den = rpool.tile([1, 1], f32, name=f"den{rep}")
    nc.vector.tensor_scalar_add(den[:, :], ex[:, :], 1.0)
    val0 = rpool.tile([1, 1], f32, name=f"val0{rep}")
    nc.vector.reciprocal(val0[:, :], den[:, :])
    val1 = rpool.tile([1, 1], f32, name=f"val1{rep}")
    nc.vector.tensor_mul(val1[:, :], ex[:, :], val0[:, :])
    vals01 = rpool.tile([1, 2], f32, name=f"vals01{rep}")
    nc.vector.tensor_copy(vals01[:, 0:1], val0[:, :])
    nc.vector.tensor_copy(vals01[:, 1:2], val1[:, :])
    vals_bc = rpool.tile([P, 2], f32, name=f"vals_bc{rep}")
    nc.gpsimd.partition_broadcast(vals_bc[:, :], vals01[:, :])

    # ---------- phase 3: expert indices into engine registers ----------
    idx_vals = []
    for k in range(TOP_K):
        v = nc.values_load(
            idx8[0:1, k : k + 1],
            engines=[ET.SP, ET.Activation, ET.Pool],
            min_val=0,
            max_val=N_EXPERTS - 1,
            # the runtime-assert path crashes the axon worker; bounds are
            # still enforced at compile time via min/max_val
            skip_runtime_bounds_check=True,
        )
        idx_vals.append(v)

    # ---------- phase 4a: up projections for both experts ----------
    dma_engines = [nc.sync, nc.scalar]
    ndma = 0
    h_tiles = []
    bd_tiles = []
    for k in range(TOP_K):
        iv = idx_vals[k]
        row = bass.ts(iv, P)  # rows e*128 .. e*128+127

        bup_sb = hpool.tile([P, SJ], f32, name=f"bup{rep}_{k}", tag="bup")
        nc.gpsimd.dma_start(bup_sb[:, :], bup_d[row, :])
        bd = rpool.tile([1, VOCAB], f32, name=f"bd{rep}_{k}", tag=f"bd{k}")
        nc.gpsimd.dma_start(bd[:, :], bdown_d[bass.ds(iv, 1), :])
        bd_tiles.append(bd)

        # h[128,4] += Wup_blk.T @ c_chunk
        ph = psr.tile([P, SJ], f32, name=f"ph{rep}_{k}", tag="ps_small")
        wup_t = []
        for g in range(WUP_CHUNKS):
            wt = wpool.tile([P, CHUNK], f32, name=f"wu{rep}_{k}_{g}", tag="w")
            eng = dma_engines[ndma % 2]
            ndma += 1
            eng.dma_start(wt[:, :], wup_d[row, g * CHUNK : (g + 1) * CHUNK])
            wup_t.append(wt)
        # one accumulation group at a time per PSUM bank: finish column j
        # over all 16 H-chunks before starting column j+1
        for j in range(SJ):
            for c in range(HC):
                wt = wup_t[c // 4]
                base = (c % 4) * SHARD
                nc.tensor.matmul(
                    ph[:, j : j + 1],
                    wt[:, base + j * P : base + (j + 1) * P],
                    c_sb[:, c : c + 1],
                    start=(c == 0),
                    stop=(c == HC - 1),
                )

        if dbg_d is not None:
            ws = rpool.tile([1, 8], f32, name=f"ws{rep}_{k}", tag=f"ws{k}")
            nc.vector.tensor_copy(ws[:, 0:4], wup_t[0][0:1, 0:4])
            tensors.setdefault("wsamp", []).append(ws)

        # h = val_k * gelu(h + bup)
        h_sb = hpool.tile([P, SJ], f32, name=f"h{rep}_{k}", tag="h")
        nc.vector.tensor_add(h_sb[:, :], ph[:, :], bup_sb[:, :])
        nc.scalar.activation(h_sb[:, :], h_sb[:, :], AF.Gelu)
        nc.vector.tensor_scalar_mul(h_sb[:, :], h_sb[:, :], vals_bc[:, k : k + 1])
        h_tiles.append(h_sb)

    # release the 2-bank routing/up PSUM pool before the 8-bank out pool
    psr_ctx.__exit__(None, None, None)
    pso_ctx = tc.tile_pool(name=f"pso{rep}", bufs=1, space="PSUM")
    pso = pso_ctx.__enter__()
    pout = pso.tile([1, VOCAB], f32, name=f"pout{rep}")

    # bias first: out = val0*bdown[i0] + val1*bdown[i1]  (K=1 matmuls)
    vsrc = [val0, val1]
    for n in range(NB):
        for k in range(TOP_K):
            nc.tensor.matmul(
                pout[:, n * 512 : (n + 1) * 512],
                vsrc[k][0:1, 0:1],
                bd_tiles[k][0:1, n * 512 : (n + 1) * 512],
                start=(k == 0),
                stop=False,
            )

    # ---------- phase 4b: down projections ----------
    for k in range(TOP_K):
        iv = idx_vals[k]
        row = bass.ts(iv, P)
        h_sb = h_tiles[k]
        wdn_t = []
        for g in range(WDN_CHUNKS):
            wt = wpool.tile([P, CHUNK], f32, name=f"wd{rep}_{k}_{g}", tag="w")
            eng = dma_engines[ndma % 2]
            ndma += 1
            eng.dma_start(wt[:, :], wdown_d[row, g * CHUNK : (g + 1) * CHUNK])
            wdn_t.append(wt)
            if dbg_d is not None and g == 0:
                nc.vector.tensor_copy(
                    tensors["wsamp"][k][0:1, 4:8], wt[0:1, 0:4]
                )
        for j in range(SJ):
            for n in range(NB):
                wt = wdn_t[j * 2 + n // 4]
                base = (n % 4) * 512
                nc.tensor.matmul(
                    pout[:, n * 512 : (n + 1) * 512],
                    h_sb[:, j : j + 1],
                    wt[:, base : base + 512],
                    start=False,
                    stop=(k == TOP_K - 1 and j == SJ - 1),
                )

    # ---------- phase 5: partial -> DRAM, AllReduce, output ----------
    out_sb = rpool.tile([1, VOCAB], f32, name=f"out_sb{rep}", tag="out_sb")
    nc.vector.tensor_copy(out_sb[:, :], pout[:, :])
    ar_in = dpool.tile([1, VOCAB], f32, name=f"ar_in{rep}")
    ar_out = dpool.tile([1, VOCAB], f32, addr_space="Shared", name=f"ar_out{rep}")
    nc.sync.dma_start(ar_in[:, :], out_sb[:, :])
    nc.gpsimd.collective_compute(
        "AllReduce",
        mybir.AluOpType.add,
        replica_groups=[list(range(N_CORES))],
        ins=[ar_in.opt()],
        outs=[ar_out.opt()],
    )
    nc.sync.dma_start(out_d[:, :], ar_out[:, :])
    if dbg_d is not None:
        dbg_sb = rpool.tile([1, 64], f32, name=f"dbg_sb{rep}", tag="dbg")
        nc.vector.memset(dbg_sb[:, :], 0.0)
        nc.vector.tensor_copy(dbg_sb[:, 0:16], logits[:, :])
        nc.vector.tensor_copy(dbg_sb[:, 16:24], max8[:, :])
        nc.vector.tensor_copy(dbg_sb[:, 24:32], idx8[:, :])  # uint->f32 convert
        nc.vector.tensor_copy(dbg_sb[:, 32:34], vals01[:, :])
        nc.vector.tensor_copy(dbg_sb[:, 34:38], h_tiles[0][0:1, 0:4])
        nc.vector.tensor_copy(dbg_sb[:, 38:42], h_tiles[1][0:1, 0:4])
        nc.vector.tensor_copy(dbg_sb[:, 42:50], tensors["wsamp"][0][:, :])
        nc.vector.tensor_copy(dbg_sb[:, 50:58], tensors["wsamp"][1][:, :])
        nc.vector.tensor_copy(dbg_sb[:, 58:60], vals_bc[0:1, 0:2])
        # debug build: clobber the first 64 output elements with diagnostics
        nc.sync.dma_start(dbg_d[0:1, 0:64], dbg_sb[:, :])
    pso_ctx.__exit__(None, None, None)


def _build(repeat=1, debug=False):
    """Build + compile the Bass module once per process."""
    key = ("nc", repeat, debug)
    if key in _CACHE:
        return _CACHE[key], _CACHE["names"]

    import concourse.bacc as bacc
    import concourse.mybir as mybir
    import concourse.tile as tile

    f32 = mybir.dt.float32

    nc = bacc.Bacc(
        "TRN2",
        target_bir_lowering=False,
        debug=False,
        enable_asserts=False,
        num_devices=N_CORES,
    )

    # ---- external inputs (pre-swizzled on host, see kernel()) ----
    statesT_d = nc.dram_tensor("statesT", [P, HC * CELLS], f32, kind="ExternalInput").ap()
    wr_d = nc.dram_tensor("wr", [P, HC * N_EXPERTS], f32, kind="ExternalInput").ap()
    br_d = nc.dram_tensor("br", [1, N_EXPERTS], f32, kind="ExternalInput").ap()
    wup_d = nc.dram_tensor("wup", [N_EXPERTS * P, HC * SHARD], f32, kind="ExternalInput").ap()
    wdown_d = nc.dram_tensor("wdown", [N_EXPERTS * P, SJ * VOCAB], f32, kind="ExternalInput").ap()
    bup_d = nc.dram_tensor("bup", [N_EXPERTS * P, SJ], f32, kind="ExternalInput").ap()
    bdown_d = nc.dram_tensor("bdown", [N_EXPERTS, VOCAB], f32, kind="ExternalInput").ap()
    out_d = nc.dram_tensor("out", [1, VOCAB], f32, kind="ExternalOutput").ap()

    with tile.TileContext(nc) as tc:
        with (
            tc.tile_pool(name="const", bufs=2) as cpool,
            tc.tile_pool(name="route", bufs=2) as rpool,
            tc.tile_pool(name="wchunk", bufs=12) as wpool,
            tc.tile_pool(name="hpool", bufs=2) as hpool,
            tc.tile_pool(name="dram", bufs=1, space="DRAM") as dpool,
        ):
            tensors = dict(
                dram=(statesT_d, wr_d, br_d, wup_d, wdown_d, bup_d, bdown_d, out_d),
                pools=(cpool, rpool, wpool, hpool, dpool),
                dbg=(out_d if debug else None),
            )
            for rep in range(repeat):
                _emit_body(nc, tc, tensors, rep)

    nc.compile()
    names = dict(
        inputs=["statesT", "wr", "br", "wup", "wdown", "bup", "bdown"],
        output="out",
    )
    _CACHE[key] = nc
    _CACHE["names"] = names
    return nc, names


def _stage_inputs(states, Wr, br, Wup, bup, Wdown, bdown):
    """Swizzle full inputs into the per-core layouts the device kernel expects."""
    f = np.float32
    states = np.asarray(states, f)
    Wr = np.asarray(Wr, f)
    br = np.asarray(br, f)
    Wup = np.asarray(Wup, f)
    bup = np.asarray(bup, f)
    Wdown = np.asarray(Wdown, f)
    bdown = np.asarray(bdown, f)

    # [p, c*64+t] = states[t, c*128+p]
    statesT = np.ascontiguousarray(
        states.T.reshape(HC, P, CELLS).transpose(1, 0, 2).reshape(P, HC * CELLS)
    )
    wr = np.ascontiguousarray(
        Wr.reshape(HC, P, N_EXPERTS).transpose(1, 0, 2).reshape(P, HC * N_EXPERTS)
    )
    br2 = br.reshape(1, N_EXPERTS)

    in_maps = []
    for core in range(N_CORES):
        s0 = core * SHARD
        # Wup[e][:, shard] -> [e*128+p, c*SHARD+m]
        wu = (
            Wup[:, :, s0 : s0 + SHARD]
            .reshape(N_EXPERTS, HC, P, SHARD)
            .transpose(0, 2, 1, 3)
            .reshape(N_EXPERTS * P, HC * SHARD)
        )
        # Wdown[e][shard, :] -> [e*128+p, j*V+v]
        wd = (
            Wdown[:, s0 : s0 + SHARD, :]
            .reshape(N_EXPERTS, SJ, P, VOCAB)
            .transpose(0, 2, 1, 3)
            .reshape(N_EXPERTS * P, SJ * VOCAB)
        )
        bu = (
            bup[:, s0 : s0 + SHARD]
            .reshape(N_EXPERTS, SJ, P)
            .transpose(0, 2, 1)
            .reshape(N_EXPERTS * P, SJ)
        )
        bd = bdown if core == 0 else np.zeros_like(bdown)
        in_maps.append(
            {
                "statesT": statesT,
                "wr": wr,
                "br": br2,
                "wup": np.ascontiguousarray(wu),
                "wdown": np.ascontiguousarray(wd),
                "bup": np.ascontiguousarray(bu),
                "bdown": np.ascontiguousarray(bd),
            }
        )
    return in_maps


def run(trace=False, tmpdir=None, **inputs):
    """Run the kernel; returns (output[4096], exec_time_ns or None)."""
    from concourse.bass_utils import run_bass_kernel_spmd

    nc, names = _build(debug=trace == "debug")
    in_maps = _stage_inputs(**inputs)
    res = run_bass_kernel_spmd(
        nc, in_maps, core_ids=list(range(N_CORES)),
        trace=bool(trace) and trace != "debug", tmpdir=tmpdir,
    )
    out = np.asarray(res.results[0][names["output"]], np.float32).reshape(VOCAB)
    run.dbg = [r.get("out") for r in res.results]
    return out, res.exec_time_ns


def kernel(**inputs) -> np.ndarray:
    out, _ = run(trace=False, **inputs)
    return out

